# revision 12
# baseline (speedup 1.0000x reference)
"""Trainium2 Bass kernel for nn_MultiHeadAttention_88055419502796.

Full attention (t=1024) with clipped relative-position bias (window +-10).
Sharding: 8 cores = 4 batches x 2 head-groups (6 heads each). Each core:
  - QKV projection (PE, bf16 in / f32 psum)
  - per 128-query block: scores = qs^T k  [t-part, s-free]
  - rel-k bias: tiny matmul T = qs^T @ embA  [128,20] (col j<19: band r=19-j
    minus emb[0] (softmax shift), col 19: g = emb[20]-emb[0]);
    expanded row buffer E = [g x127 | band x19 | 0 x127] round-tripped through
    DRAM with a skewed (diagonal) access pattern -> rectangular bias tile,
    DVE-added into scores.  Uniform far-past region handled via per-partition
    bias on a split exp() call.  Far-future region is 0 by the softmax shift.
  - exp via ACT (no max subtraction; scores ~ N(0,1)), accum_out = rowsum
  - p transposed via XBAR DMA-transpose; PV matmuls (lhsT = p^T, rhs = v^T)
  - rel-v: band of p extracted by the same DRAM skew trick; a/b columns from
    masked reduce + suffix-block matmul with ones; G @ emb_v into PV psum
  - normalize by 1/rowsum, transpose att, output projection -> partial [768,1024]
Host sums the two head-group partials per batch and adds exact bias terms.
"""

import numpy as np
import ml_dtypes
from contextlib import ExitStack

import concourse.bass as bass
import concourse.bacc as bacc
import concourse.mybir as mybir
import concourse.tile as tile
from concourse.bass_utils import run_bass_kernel_spmd

FP32 = mybir.dt.float32
BF16 = mybir.dt.bfloat16
BF = ml_dtypes.bfloat16
AX = mybir.AxisListType
ALU = mybir.AluOpType
ACTF = mybir.ActivationFunctionType

C, H, D, T, WIN = 768, 12, 64, 1024, 10
HPC = 6            # heads per core
NB = T // 128      # 8 query blocks

# DRAM scratch geometry
E_W = 273          # [g x127 | band x19 | zero x127]
PB_STRIDE = 387    # p-slice scratch row stride (max slice 384 + pad)
PB_HEAD = 16
PB_TOTAL = PB_HEAD + 128 * PB_STRIDE + 112   # == 128*388 exactly


def _build_program(debug=False):
    nc = bacc.Bacc("TRN2", target_bir_lowering=False, debug=False, num_devices=8)

    x6 = nc.dram_tensor("x6", [6, 128, T], BF16, kind="ExternalInput").ap()
    wt = nc.dram_tensor("wt", [6, 128, 1152], BF16, kind="ExternalInput").ap()
    wot = nc.dram_tensor("wot", [3, 128, 768], BF16, kind="ExternalInput").ap()
    embat = nc.dram_tensor("embat", [128, 20], BF16, kind="ExternalInput").ap()
    embv = nc.dram_tensor("embv", [21, 64], BF16, kind="ExternalInput").ap()
    futmask = nc.dram_tensor("futmask", [128, 384], BF16, kind="ExternalInput").ap()
    maskbf = nc.dram_tensor("maskbf", [128, 19], BF16, kind="ExternalInput").ap()
    maskbl = nc.dram_tensor("maskbl", [128, 19], BF16, kind="ExternalInput").ap()
    outp = nc.dram_tensor("outp", [6, 128, T], FP32, kind="ExternalOutput").ap()
    dbg = {}
    if debug:
        for nm, shp, dt_ in [("d_q", [128, T], BF16), ("d_k", [128, T], BF16),
                             ("d_v", [128, T], BF16), ("d_t", [128, 20], FP32),
                             ("d_e", [128, E_W], BF16), ("d_bmix", [128, 146], FP32),
                             ("d_p", [128, T], BF16), ("d_pt", [128, T], BF16),
                             ("d_gpad", [128, 128], BF16), ("d_pv", [128, 65], FP32),
                             ("d_scal", [128, 8], FP32), ("d_vaug", [128, 512], BF16),
                             ("d_att", [128, 3 * T], BF16), ("d_scores", [128, T], FP32)]:
            dbg[nm] = nc.dram_tensor(nm, shp, dt_, kind="ExternalOutput").ap()

    e_scr = [nc.dram_tensor(f"e_scr{i}", [128 * E_W], BF16, kind="Internal")
             for i in range(2)]
    pb_scr = [nc.dram_tensor(f"pb_scr{i}", [PB_TOTAL], BF16, kind="Internal")
              for i in range(2)]

    with tile.TileContext(nc) as tc, ExitStack() as ctx:
        consts = ctx.enter_context(tc.tile_pool(name="consts", bufs=1))
        ps_scores = ctx.enter_context(
            tc.tile_pool(name="ps_scores", bufs=2, space=bass.MemorySpace.PSUM))
        ps_pv = ctx.enter_context(
            tc.tile_pool(name="ps_pv", bufs=2, space=bass.MemorySpace.PSUM))
        ps_f = ctx.enter_context(
            tc.tile_pool(name="ps_f", bufs=2, space=bass.MemorySpace.PSUM))
        wk = ctx.enter_context(tc.tile_pool(name="work", bufs=4))
        wk2 = ctx.enter_context(tc.tile_pool(name="work2", bufs=4))

        # ---- persistent SBUF ----
        x_sb = consts.tile([128, 6 * T], BF16, tag="x")
        wt_sb = consts.tile([128, 6 * 1152], BF16, tag="wt")
        wot_sb = consts.tile([128, 3 * 768], BF16, tag="wot")
        embat_sb = consts.tile([128, 20], BF16, tag="embat")
        embv_sb = consts.tile([21, 64], BF16, tag="embv")
        futmask_sb = consts.tile([128, 384], BF16, tag="futmask")
        maskbf_sb = consts.tile([128, 19], BF16, tag="maskbf")
        maskbl_sb = consts.tile([128, 19], BF16, tag="maskbl")
        qkv_sb = consts.tile([128, 9 * T], BF16, tag="qkv")
        vaug_sb = consts.tile([128, HPC * 512], BF16, tag="vaug")
        attT_sb = consts.tile([128, 3 * T], BF16, tag="attT")
        ones_sb = consts.tile([128, 1], BF16, tag="ones")
        zeros_sb = consts.tile([128, 388], BF16, tag="zeros")

        for i in range(6):
            nc.sync.dma_start(x_sb[:, i * T:(i + 1) * T], x6[i])
            nc.sync.dma_start(wt_sb[:, i * 1152:(i + 1) * 1152], wt[i])
        for i in range(3):
            nc.sync.dma_start(wot_sb[:, i * 768:(i + 1) * 768], wot[i])
        nc.sync.dma_start(embat_sb[:], embat)
        nc.sync.dma_start(embv_sb[:], embv)
        nc.sync.dma_start(futmask_sb[:], futmask)
        nc.sync.dma_start(maskbf_sb[:], maskbf)
        nc.sync.dma_start(maskbl_sb[:], maskbl)
        nc.gpsimd.memset(ones_sb[:], 1.0)
        nc.gpsimd.memset(zeros_sb[:], 0.0)
        # zero the p-band scratch (garbage there is masked but NaN*0 = NaN)
        for i in range(2):
            nc.sync.dma_start(
                bass.AP(pb_scr[i], 0, [[1, PB_TOTAL]]), zeros_sb[:])

        # ---- QKV projection ----
        for m in range(3):
            for ob in range(3):
                ps = ps_scores.tile([128, T], FP32, tag="ps")
                for kc in range(6):
                    lhsT = wt_sb[:, kc * 1152 + m * 384 + ob * 128:
                                 kc * 1152 + m * 384 + (ob + 1) * 128]
                    for hf in range(2):
                        nc.tensor.matmul(
                            ps[:, hf * 512:(hf + 1) * 512], lhsT,
                            x_sb[:, kc * T + hf * 512: kc * T + (hf + 1) * 512],
                            start=(kc == 0), stop=(kc == 5))
                nc.scalar.copy(
                    qkv_sb[:, m * 3072 + ob * T: m * 3072 + (ob + 1) * T], ps[:])

        # ---- v transposes -> vaug ----
        for h in range(HPC):
            r0 = (h % 2) * 64
            cb = 6144 + (h // 2) * T
            for b in range(8):
                nc.sync.dma_start(
                    vaug_sb[:, h * 512 + b * 64: h * 512 + (b + 1) * 64],
                    qkv_sb[r0:r0 + 64, cb + b * 128: cb + (b + 1) * 128],
                    transpose=True)

        if debug:
            nc.sync.dma_start(dbg["d_q"], qkv_sb[:, 0:T])
            nc.sync.dma_start(dbg["d_k"], qkv_sb[:, 3072:3072 + T])
            nc.sync.dma_start(dbg["d_v"], qkv_sb[:, 6144:6144 + T])
            nc.sync.dma_start(dbg["d_vaug"], vaug_sb[:, 0:512])

        # ---- attention ----
        for h in range(HPC):
            r0 = (h % 2) * 64
            qc = (h // 2) * T
            kc_ = 3072 + (h // 2) * T
            for j in range(NB):
                t0 = j * 128
                q_blk = qkv_sb[r0:r0 + 64, qc + t0: qc + t0 + 128]

                psf = ps_f.tile([128, 20], FP32, tag="psf")
                nc.tensor.matmul(psf[:], q_blk, embat_sb[r0:r0 + 64, :], start=True, stop=True)

                ps_s = ps_scores.tile([128, T], FP32, tag="ps")
                for hf in range(2):
                    nc.tensor.matmul(
                        ps_s[:, hf * 512:(hf + 1) * 512], q_blk,
                        qkv_sb[r0:r0 + 64, kc_ + hf * 512: kc_ + (hf + 1) * 512],
                        start=True, stop=True)

                tsb = wk.tile([128, 20], FP32, tag="tsb")
                nc.vector.tensor_copy(tsb[:], psf[:])
                dbg_this = debug and h == 0 and j == 3
                if dbg_this:
                    nc.sync.dma_start(dbg["d_t"], tsb[:])

                # build E = [g x127 | band x19 | 0 x127], bf16
                e_sb = wk.tile([128, E_W], BF16, tag="esb")
                nc.gpsimd.tensor_scalar_add(
                    e_sb[:, 0:127], zeros_sb[:, 0:127], tsb[:, 19:20])
                nc.gpsimd.tensor_copy(e_sb[:, 127:146], tsb[:, 0:19])
                nc.gpsimd.memset(e_sb[:, 146:273], 0.0)

                esc = e_scr[j % 2]
                nc.sync.dma_start(
                    bass.AP(esc, 0, [[E_W, 128], [1, E_W]]), e_sb[:])
                # skewed read: addr = i*272 + jr + off
                if j == 0:
                    ew, eoff, dlo = 137, 127 + 9, 0
                elif j == NB - 1:
                    ew, eoff, dlo = 137, 127, t0 - 9
                else:
                    ew, eoff, dlo = 146, 127, t0 - 9
                bmix = wk.tile([128, 146], FP32, tag="bmix")
                nc.gpsimd.dma_start(
                    bmix[:, 0:ew], bass.AP(esc, eoff, [[E_W - 1, 128], [1, ew]]))
                if dbg_this:
                    nc.sync.dma_start(dbg["d_e"], e_sb[:])
                    nc.sync.dma_start(dbg["d_bmix"], bmix[:])
                nc.vector.tensor_add(
                    ps_s[:, dlo:dlo + ew], ps_s[:, dlo:dlo + ew], bmix[:, 0:ew])
                if dbg_this:
                    sc_dbg = wk.tile([128, T], FP32, tag="scdbg")
                    nc.vector.tensor_copy(sc_dbg[:], ps_s[:])
                    nc.sync.dma_start(dbg["d_scores"], sc_dbg[:])

                # exp (split: far-past columns get per-partition bias g)
                p_sb = wk.tile([128, T], BF16, tag="p")
                scal = wk.tile([128, 10], FP32, tag="scal")
                c0 = t0 - 9 if j >= 1 else 0
                if c0 > 0:
                    nc.scalar.activation(
                        p_sb[:, 0:c0], ps_s[:, 0:c0], ACTF.Exp,
                        bias=tsb[:, 19:20], accum_out=scal[:, 0:1])
                    nc.scalar.activation(
                        p_sb[:, c0:T], ps_s[:, c0:T], ACTF.Exp,
                        accum_out=scal[:, 1:2])
                    nc.vector.tensor_add(scal[:, 2:3], scal[:, 0:1], scal[:, 1:2])
                else:
                    nc.scalar.activation(
                        p_sb[:], ps_s[:], ACTF.Exp, accum_out=scal[:, 2:3])

                if dbg_this:
                    nc.sync.dma_start(dbg["d_p"], p_sb[:])
                # transpose p
                pt_sb = wk.tile([128, T], BF16, tag="pt")
                for b in range(8):
                    nc.sync.dma_start(
                        pt_sb[:, b * 128:(b + 1) * 128],
                        p_sb[:, b * 128:(b + 1) * 128], transpose=True)

                # PV + suffix matmuls
                pv = ps_pv.tile([128, 65], FP32, tag="pv")
                for b in range(8):
                    nc.tensor.matmul(
                        pv[:, 0:64], pt_sb[:, b * 128:(b + 1) * 128],
                        vaug_sb[:, h * 512 + b * 64: h * 512 + (b + 1) * 64],
                        start=(b == 0), stop=(b == 7))
                # suffix sum over fully-future blocks on ACT
                if j <= 5:
                    sw = T - (j + 2) * 128
                    sfx = wk2.tile([128, 768], BF16, tag="sfx")
                    nc.scalar.activation(
                        sfx[:, 0:sw], p_sb[:, (j + 2) * 128:T], ACTF.Identity,
                        accum_out=scal[:, 8:9])

                # fut_red: masked reduce over the 3-block slice
                if j == 0:
                    psl, msl, wp = (0, 256), (128, 384), 256
                elif j == NB - 1:
                    psl, msl, wp = (768, 1024), (0, 256), 256
                else:
                    psl, msl, wp = ((j - 1) * 128, (j + 2) * 128), (0, 384), 384
                fo = wk2.tile([128, 384], BF16, tag="fo")
                nc.vector.tensor_mul(fo[:, 0:wp], p_sb[:, psl[0]:psl[1]],
                                     futmask_sb[:, msl[0]:msl[1]])
                nc.vector.reduce_sum(scal[:, 3:4], fo[:, 0:wp], axis=AX.X)

                # band of p via DRAM skew
                pbs = pb_scr[j % 2]
                nc.sync.dma_start(
                    bass.AP(pbs, PB_HEAD, [[PB_STRIDE, 128], [1, wp]]),
                    p_sb[:, psl[0]:psl[1]])
                g_pad = wk2.tile([128, 128], BF16, tag="gpad")
                boff = PB_HEAD - 9 if j == 0 else PB_HEAD + 119
                nc.sync.dma_start(
                    g_pad[:, 0:19],
                    bass.AP(pbs, boff, [[PB_STRIDE + 1, 128], [1, 19]]))
                if j == 0:
                    nc.vector.tensor_mul(g_pad[:, 0:19], g_pad[:, 0:19], maskbf_sb[:])
                elif j == NB - 1:
                    nc.vector.tensor_mul(g_pad[:, 0:19], g_pad[:, 0:19], maskbl_sb[:])
                nc.vector.reduce_sum(scal[:, 4:5], g_pad[:, 0:19], axis=AX.X)

                # a, b columns
                if j <= 5:
                    nc.vector.tensor_add(scal[:, 5:6], scal[:, 3:4], scal[:, 8:9])
                else:
                    nc.vector.tensor_copy(scal[:, 5:6], scal[:, 3:4])
                nc.vector.tensor_sub(scal[:, 6:7], scal[:, 2:3], scal[:, 5:6])
                nc.vector.tensor_sub(scal[:, 6:7], scal[:, 6:7], scal[:, 4:5])
                nc.vector.tensor_copy(g_pad[:, 19:20], scal[:, 5:6])
                nc.vector.tensor_copy(g_pad[:, 20:21], scal[:, 6:7])
                nc.gpsimd.memset(g_pad[:, 21:128], 0.0)

                gt = wk2.tile([128, 128], BF16, tag="gt")
                nc.sync.dma_start(gt[:], g_pad[:], transpose=True)
                nc.tensor.matmul(pv[:, 0:64], gt[0:21, :], embv_sb[:],
                                 start=False, stop=True, skip_group_check=True)

                if dbg_this:
                    nc.sync.dma_start(dbg["d_pt"], pt_sb[:])
                    nc.sync.dma_start(dbg["d_gpad"], g_pad[:])
                    nc.sync.dma_start(dbg["d_scal"][:, 0:7], scal[:, 0:7])
                    pv_dbg = wk.tile([128, 65], FP32, tag="pvdbg")
                    nc.vector.tensor_copy(pv_dbg[:, 0:64], pv[:, 0:64])
                    nc.gpsimd.memset(pv_dbg[:, 64:65], 0.0)
                    nc.sync.dma_start(dbg["d_pv"], pv_dbg[:])
                # normalize + transpose att
                nc.vector.reciprocal(scal[:, 7:8], scal[:, 2:3])
                att_pad = wk2.tile([128, 128], BF16, tag="attpad")
                nc.vector.tensor_scalar_mul(
                    att_pad[:, 0:64], pv[:, 0:64], scal[:, 7:8])
                nc.gpsimd.memset(att_pad[:, 64:128], 0.0)
                att_t = wk2.tile([128, 128], BF16, tag="attt")
                nc.sync.dma_start(att_t[:], att_pad[:], transpose=True)
                nc.vector.tensor_copy(
                    attT_sb[r0:r0 + 64, (h // 2) * T + t0:
                            (h // 2) * T + t0 + 128], att_t[0:64, :])

        if debug:
            nc.sync.dma_start(dbg["d_att"], attT_sb[:])
        # ---- output projection ----
        for ob in range(6):
            ps = ps_scores.tile([128, T], FP32, tag="ps")
            for kc in range(3):
                lhsT = wot_sb[:, kc * 768 + ob * 128: kc * 768 + (ob + 1) * 128]
                for hf in range(2):
                    nc.tensor.matmul(
                        ps[:, hf * 512:(hf + 1) * 512], lhsT,
                        attT_sb[:, kc * T + hf * 512: kc * T + (hf + 1) * 512],
                        start=(kc == 0), stop=(kc == 2))
            osb = wk.tile([128, T], FP32, tag="osb")
            nc.vector.tensor_copy(osb[:], ps[:])
            nc.sync.dma_start(outp[ob], osb[:])

    nc.compile()
    return nc


_NC = None
TRACE = False
LAST_RESULT = None


def _run(nc, in_maps):
    global LAST_RESULT
    res = run_bass_kernel_spmd(nc, in_maps, core_ids=list(range(8)), trace=TRACE)
    LAST_RESULT = res
    return res


def _host_consts():
    i = np.arange(128)[:, None]
    c = np.arange(384)[None, :]
    m = np.arange(19)[None, :]
    futmask = (c >= i + 138).astype(BF)
    maskbf = ((i + m - 9) >= 0).astype(BF)
    maskbl = ((i + m + 119) <= 255).astype(BF)
    return futmask, maskbf, maskbl


def kernel(x, wq, bq, wk, bk, wv, bv, wo, bo, emb_rel_k, emb_rel_v):
    global _NC
    if _NC is None:
        _NC = _build_program()
    nc = _NC

    x = np.asarray(x, np.float32)
    scale = np.float32(D ** -0.5)
    ek = np.asarray(emb_rel_k, np.float32)
    ev = np.asarray(emb_rel_v, np.float32)

    embat = np.zeros((128, 20), np.float32)      # col j<19: emb[19-j]-emb[0], both halves
    embat[0:64, 0:19] = (ek[19:0:-1] - ek[0]).T
    embat[0:64, 19] = ek[20] - ek[0]
    embat[64:128] = embat[0:64]
    embv = np.zeros((21, 64), np.float32)
    embv[0:19] = ev[19:0:-1]
    embv[19] = ev[0]
    embv[20] = ev[20]
    futmask, maskbf, maskbl = _host_consts()

    in_maps = []
    for core in range(8):
        bi, hg = core // 2, core % 2
        rows = slice(hg * 384, (hg + 1) * 384)
        wT = np.concatenate([
            (np.asarray(wq, np.float32)[rows] * scale).T,
            np.asarray(wk, np.float32)[rows].T,
            np.asarray(wv, np.float32)[rows].T], axis=1)     # [768, 1152]
        in_maps.append({
            "x6": np.ascontiguousarray(x[bi].reshape(6, 128, T)).astype(BF),
            "wt": np.ascontiguousarray(wT.reshape(6, 128, 1152)).astype(BF),
            "wot": np.ascontiguousarray(
                np.asarray(wo, np.float32)[:, rows].T.reshape(3, 128, 768)).astype(BF),
            "embat": embat.astype(BF),
            "embv": embv.astype(BF),
            "futmask": futmask,
            "maskbf": maskbf,
            "maskbl": maskbl,
        })

    global LAST_RESULT
    res = _run(nc, in_maps)
    out = np.zeros((4, C, T), np.float32)
    for core in range(8):
        out[core // 2] += res.results[core]["outp"].reshape(C, T)
    bv_f = np.asarray(bv, np.float32)
    out += (np.asarray(wo, np.float32) @ bv_f)[None, :, None]
    out += np.asarray(bo, np.float32)[None, :, None]
    return out
